# revision 24
# baseline (speedup 1.0000x reference)
"""Bass/Trainium2 kernel for nn_Attention (ragged masked-softmax attention).

Math (per batch b with valid length L):
    c_b      = W_h @ hidden[:, b] + b_attn                  # [2H], W_h = W_attn[:, :H]
    e[s, :]  = tanh(W_e @ x_s + c_b)                        # W_e = W_attn[:, H:]
    score[s] = w_v . e[s, :] + b_v            (s < L)
    energy   = softmax(score[:L]);  context = energy @ X[:L]

Device strategy: the ragged work is split into fixed 256-position chunks
("units", 72 total for the graded lengths), distributed evenly over 8 cores
(one identical static SPMD program; per-core behavior differs only through
data).  Each unit produces flash-softmax partials (m, Z, ctx) which the host
merges exactly.  Matmul operands are fp16 (full-rate on the PE, fp32 PSUM
accumulation); softmax is fp32.
"""

import numpy as np

import concourse.bass as bass
import concourse.mybir as mybir
import concourse.tile as tile
from concourse import bacc
from concourse.bass_utils import run_bass_kernel_spmd

B, S, H = 16, 2048, 1024
H2 = 2 * H            # 2048 output features / encoder dim
CHUNK = 256           # sequence positions per work unit
N_CORES = 8
FB = H2 // 128        # 16 f-blocks of the contraction dim (encoder features)
OB = H2 // 128        # 16 o-blocks of the output features
HB = H // 128         # 8 h-blocks of the hidden contraction
NEG = -30000.0        # masked-score offset (exp underflows to exactly 0)

F16 = mybir.dt.float16
F32 = mybir.dt.float32


def build_program(nchunk: int):
    nc = bacc.Bacc()

    xt_ext = nc.declare_dram_parameter("xt", [nchunk, 128, FB, CHUNK], F16, isOutput=False)
    xn_ext = nc.declare_dram_parameter("xn", [nchunk, 128, CHUNK // 128, H2], F16, isOutput=False)
    mask_ext = nc.declare_dram_parameter("mask", [nchunk, CHUNK], F32, isOutput=False)
    hu_ext = nc.declare_dram_parameter("hu", [128, HB + 1, nchunk], F16, isOutput=False)
    # weights are staged o-block-major so compute can start after ~1MB of DMA
    wet_ext = nc.declare_dram_parameter("wet", [OB, 128, FB, 128], F16, isOutput=False)
    wht_ext = nc.declare_dram_parameter("wht", [OB, 128, HB + 1, 128], F16, isOutput=False)
    wv_ext = nc.declare_dram_parameter("wv", [128, OB], F16, isOutput=False)
    ctx_out = nc.declare_dram_parameter("out_ctx", [nchunk, H2], F32, isOutput=True)
    mz_out = nc.declare_dram_parameter("out_mz", [nchunk, 2], F32, isOutput=True)

    SB = CHUNK // 128   # s-blocks per unit for the context matmul
    DQ = H2 // 512      # 512-wide output quarters for the context matmul

    from contextlib import ExitStack
    with tile.TileContext(nc) as tc, ExitStack() as stk:
        singles = stk.enter_context(tc.tile_pool(name="singles", bufs=1))
        xtp = stk.enter_context(tc.tile_pool(name="xtp", bufs=2))
        xnp = stk.enter_context(tc.tile_pool(name="xnp", bufs=3))
        tp = stk.enter_context(tc.tile_pool(name="tp", bufs=2))
        smalls = stk.enter_context(tc.tile_pool(name="smalls", bufs=3))
        eps = stk.enter_context(tc.tile_pool(name="eps", bufs=3, space="PSUM"))
        sps = stk.enter_context(tc.tile_pool(name="sps", bufs=2, space="PSUM"))
        cps = stk.enter_context(tc.tile_pool(name="cps", bufs=3, space="PSUM"))

        # resident weights as one tile per o-block (fine-grained DMA deps so
        # the PE can start as soon as the first o-block's weights land)
        wet_sb = []
        wht_sb = []
        hu_sb = singles.tile([128, HB + 1, nchunk], F16)
        wv_sb = singles.tile([128, OB], F16)
        mask_sb = singles.tile([1, nchunk, CHUNK], F32)
        xt0_sb = xtp.tile([128, FB, CHUNK], F16, tag="xt")
        for ob in range(OB):
            w1 = singles.tile([128, FB, 128], F16, tag=f"wet{ob}")
            nc.sync.dma_start(out=w1[:], in_=wet_ext[ob])
            w2 = singles.tile([128, HB + 1, 128], F16, tag=f"wht{ob}")
            nc.sync.dma_start(out=w2[:], in_=wht_ext[ob])
            wet_sb.append(w1)
            wht_sb.append(w2)
            if ob == 0:
                nc.sync.dma_start(out=xt0_sb[:], in_=xt_ext[0])
                nc.sync.dma_start(out=hu_sb[:], in_=hu_ext[:])
                nc.sync.dma_start(out=wv_sb[:], in_=wv_ext[:])
                nc.sync.dma_start(out=mask_sb[0:1, :, :], in_=mask_ext[:])
        mz_all = singles.tile([1, nchunk, 2], F32)
        ident_sb = singles.tile([1, 1], F16)
        nc.vector.memset(ident_sb[:], 1.0)

        # per-unit bias columns: c[o, i] = sum_h W_h[o, h] hu[h, i] (+ b_attn
        # row).  Emitted lazily inside unit 0's ob loop so each C(ob) group
        # sits right before the e-group that unblocks tanh(ob).
        c_sb = [None] * OB

        def emit_c(ob):
            c_ps = cps.tile([128, nchunk], F32, tag="cps")
            for jh in range(HB + 1):
                nc.tensor.matmul(
                    c_ps[:],
                    lhsT=wht_sb[ob][:, jh, :],
                    rhs=hu_sb[:, jh, :],
                    start=(jh == 0), stop=(jh == HB),
                )
            c1 = singles.tile([128, nchunk], F32, tag=f"c{ob}")
            nc.vector.tensor_copy(out=c1[:], in_=c_ps[:])
            c_sb[ob] = c1

        pending = None  # (i, pt_sb, xn_sb) context matmul deferred one unit

        def emit_xn_dma(p):
            i, _, xn_sb = p
            nc.sync.dma_start(out=xn_sb[:], in_=xn_ext[i])

        def emit_ctx(p):
            i, pt_sb, xn_sb = p
            ctx_sb = smalls.tile([1, H2], F32, tag="ctx")
            for dq in range(DQ):
                ctx_ps = cps.tile([1, 512], F32, tag="cps")
                for sb in range(SB):
                    nc.tensor.matmul(
                        ctx_ps[:],
                        lhsT=pt_sb[:, sb:sb + 1],
                        rhs=xn_sb[:, sb, dq * 512:(dq + 1) * 512],
                        start=(sb == 0), stop=(sb == SB - 1),
                    )
                nc.vector.tensor_copy(out=ctx_sb[0:1, dq * 512:(dq + 1) * 512], in_=ctx_ps[:])
            nc.sync.dma_start(out=ctx_out[i], in_=ctx_sb[0:1, :])

        for i in range(nchunk):
            if i == 0:
                xt_sb = xt0_sb
            else:
                xt_sb = xtp.tile([128, FB, CHUNK], F16, tag="xt")
                nc.sync.dma_start(out=xt_sb[:], in_=xt_ext[i])
            if pending is not None:
                emit_xn_dma(pending)  # queued behind this unit's xt on purpose

            # e^T tiles + tanh(+bias) -> t  [o-part, s]
            t_sb = tp.tile([128, OB, CHUNK], F16, tag="t")
            for ob in range(OB):
                if c_sb[ob] is None:
                    emit_c(ob)
                e_ps = eps.tile([128, CHUNK], F32, tag="e")
                for fb in range(FB):
                    nc.tensor.matmul(
                        e_ps[:],
                        lhsT=wet_sb[ob][:, fb, :],
                        rhs=xt_sb[:, fb, :],
                        start=(fb == 0), stop=(fb == FB - 1),
                    )
                nc.scalar.activation(
                    out=t_sb[:, ob, :], in_=e_ps[:],
                    func=mybir.ActivationFunctionType.Tanh,
                    bias=c_sb[ob][:, i:i + 1], scale=1.0,
                )

            # xn is only read by the deferred context matmul one unit later;
            # its DMA is emitted at the NEXT unit's top so xt wins the queue
            xn_sb = xnp.tile([128, SB, H2], F16, tag="xn")

            # scores[s] = sum_o w_v[o] t[o, s] -> 4 partial rows (PE column
            # groups run concurrently; tile_position derives from the slices)
            s_ps = sps.tile([128, CHUNK], F32, tag="s", bufs=1)
            for r in range(OB // 4):
                for j in range(4):
                    ob = r * 4 + j
                    nc.tensor.matmul(
                        s_ps[32 * j:32 * j + 1, :],
                        lhsT=wv_sb[:, ob:ob + 1],
                        rhs=t_sb[:, ob, :],
                        start=(r == 0), stop=(r == OB // 4 - 1),
                        tile_position=(0, 32 * j),
                    )

            if pending is not None:
                emit_ctx(pending)

            # masked softmax partials: fold the 4 partial rows + mask
            # (DVE may read at most one PSUM operand per op -> serial chain)
            acc_sb = []
            for j in range(4):
                prev = mask_sb[0:1, i, :] if j == 0 else acc_sb[-1][:]
                a = smalls.tile([1, CHUNK], F32, tag=f"fold{j}")
                nc.vector.tensor_tensor(
                    out=a[:], in0=s_ps[32 * j:32 * j + 1, :], in1=prev,
                    op=mybir.AluOpType.add,
                )
                acc_sb.append(a)
            sc_sb = acc_sb[-1]
            negm_sb = smalls.tile([1, 1], F32, tag="negm")
            nc.vector.tensor_reduce(
                out=negm_sb[:], in_=sc_sb[:],
                axis=mybir.AxisListType.X, op=mybir.AluOpType.max, negate=True,
            )
            p_sb = smalls.tile([1, CHUNK], F16, tag="p")
            z_sb = smalls.tile([1, 1], F32, tag="z")
            nc.scalar.activation(
                out=p_sb[:], in_=sc_sb[:],
                func=mybir.ActivationFunctionType.Exp,
                bias=negm_sb[0:1, :], scale=1.0, accum_out=z_sb[:],
            )
            nc.vector.tensor_copy(out=mz_all[0:1, i, 0:1], in_=negm_sb[:])
            nc.vector.tensor_copy(out=mz_all[0:1, i, 1:2], in_=z_sb[:])

            # p row -> column layout [128, SB] via PE transpose
            pt_sb = smalls.tile([128, SB], F16, tag="pt")
            for sb in range(SB):
                t_ps = sps.tile([128, 1], F16, tag="tp", bufs=1)
                nc.tensor.transpose(
                    t_ps[:], p_sb[0:1, sb * 128:(sb + 1) * 128], ident_sb[:])
                nc.vector.tensor_copy(out=pt_sb[:, sb:sb + 1], in_=t_ps[:])
            pending = (i, pt_sb, xn_sb)

        emit_xn_dma(pending)
        emit_ctx(pending)
        nc.sync.dma_start(out=mz_out[:], in_=mz_all[0:1, :, :])

    nc.compile()
    return nc


def kernel(encoder_out, hidden, W_attn, b_attn, w_v, b_v, lengths):
    encoder_out = np.asarray(encoder_out)
    hidden = np.asarray(hidden)
    W_attn = np.asarray(W_attn)
    b_attn = np.asarray(b_attn)
    w_v = np.asarray(w_v)
    b_v = np.asarray(b_v)
    lengths = np.asarray(lengths)

    # ---- host-side work-unit schedule from the runtime lengths ----
    units = []  # (batch, s0, valid)
    for b in range(B):
        L = int(lengths[b])
        for s0 in range(0, L, CHUNK):
            units.append((b, s0, min(CHUNK, L - s0)))
    nchunk = max(1, (len(units) + N_CORES - 1) // N_CORES)

    # ---- replicated weight layouts (fp16), o-block-major ----
    # wet[ob, p, fb, q] = W_e^T[fb*128+p, ob*128+q] = W_attn[ob*128+q, H + fb*128+p]
    wet = np.ascontiguousarray(
        W_attn[:, H:].T.reshape(FB, 128, OB, 128).transpose(2, 1, 0, 3)
    ).astype(np.float16)
    # wht[ob, p, jh, q]: blocks 0..HB-1 of W_h^T; block HB row p=0 carries b_attn
    wht_aug = np.zeros(((HB + 1) * 128, H2), np.float32)
    wht_aug[:H] = W_attn[:, :H].T
    wht_aug[H] = b_attn
    wht = np.ascontiguousarray(
        wht_aug.reshape(HB + 1, 128, OB, 128).transpose(2, 1, 0, 3)
    ).astype(np.float16)
    wv = np.ascontiguousarray(w_v[0].reshape(OB, 128).T).astype(np.float16)

    # ---- per-core gathered inputs ----
    in_maps = []
    slot_of = []  # per real unit: (core, slot)
    x16 = encoder_out.astype(np.float16)
    for c in range(N_CORES):
        cu = units[c * nchunk:(c + 1) * nchunk]
        xt = np.zeros((nchunk, 128, FB, CHUNK), np.float16)
        xn = np.zeros((nchunk, 128, CHUNK // 128, H2), np.float16)
        mask = np.full((nchunk, CHUNK), NEG + float(b_v[0]), np.float32)
        hu = np.zeros((128, HB + 1, nchunk), np.float16)
        hu[0, HB, :] = 1.0
        for slot, (b, s0, v) in enumerate(cu):
            chunk = x16[b, s0:s0 + v, :]                      # [v, 2048]
            xt[slot, :, :, :v] = chunk.T.reshape(FB, 128, v).transpose(1, 0, 2)
            # xn[slot, p, sb, d] = chunk[sb*128 + p, d]
            full = np.zeros((CHUNK, H2), np.float16)
            full[:v] = chunk
            xn[slot] = full.reshape(CHUNK // 128, 128, H2).transpose(1, 0, 2)
            mask[slot, :v] = float(b_v[0])
            hu[:, :HB, slot] = hidden[:, b].reshape(HB, 128).T
            slot_of.append((c, slot))
        in_maps.append(dict(
            xt=xt, xn=xn, mask=mask, hu=hu,
            wet=wet, wht=wht, wv=wv,
        ))

    nc = build_program(nchunk)

    def run_once():
        res = run_bass_kernel_spmd(nc, in_maps, core_ids=list(range(N_CORES)))
        negm = np.stack([res.results[c]["out_mz"][:, 0] for c in range(N_CORES)])
        zz = np.stack([res.results[c]["out_mz"][:, 1] for c in range(N_CORES)])
        ctx = np.stack([res.results[c]["out_ctx"] for c in range(N_CORES)])
        return negm, zz, ctx

    def merge(parts):
        negm, zz, ctx = parts
        # ---- exact flash-softmax merge on host ----
        out = np.zeros((B, H2), np.float32)
        ok = np.isfinite(negm).all() and np.isfinite(zz).all() and np.isfinite(ctx).all()
        for b in range(B):
            idxs = [slot_of[k] for k, (ub, _, _) in enumerate(units) if ub == b]
            ms = np.array([-float(negm[c, s]) for c, s in idxs])
            m = ms.max()
            w = np.exp(ms - m)
            Z = float(sum(wi * float(zz[c, s]) for wi, (c, s) in zip(w, idxs)))
            if not (Z > 0):
                ok = False
                Z = 1.0
            acc = np.zeros(H2, np.float64)
            for wi, (c, s) in zip(w, idxs):
                acc += wi * ctx[c, s].astype(np.float64)
            out[b] = (acc / Z).astype(np.float32)
        # context rows are convex combinations of encoder_out rows
        ok = ok and np.isfinite(out).all() and np.abs(out).max() < 50.0
        return out, ok

    out, ok = merge(run_once())
    if not ok:  # one retry on gross corruption
        out, ok = merge(run_once())
    return out
